# revision 38
# baseline (speedup 1.0000x reference)
"""DenseCRFLoss Trainium2 kernel (8-core SPMD).

Math: loss = -(WEIGHT/n) * sum_img sum_{p,q} W[p,q] * sum_k S[k,p] S[k,q]
with W = exp(-0.5*||f_p - f_q||^2), f = [xy/50, rgb/15], P = 64*64 = 4096
(inputs are first downsampled 128->64; nearest for images, 2x2-avg for segs).

Device decomposition (per core; 2 cores per image, split by row parity):
  * -0.5*d2 for a [128p x 512q] tile is ONE bf16 matmul: augmented features
    a = [f, -0.5|f|^2, 1], b = [f, 1, -0.5|f|^2], each split hi/lo into two
    bf16 vectors (28 contraction rows) so the PSUM fp32 dot is fp32-accurate.
  * W = exp(.) on ScalarE (PSUM -> SBUF, bf16 out).
  * T_J[k,q] += S_chunk^T @ W_tile on PE (contraction over the 128 p rows,
    accumulated in one PSUM bank across all row-chunks of column J).
  * per column J: slab[:, J] = T_J * S_J elementwise (DVE); host sums slab.
  * symmetry: only supertiles (I <= J) at 512x512 granularity are computed;
    off-diagonal ones use 2*S (pre-doubled bf16 weights) to count both sides.
"""

import numpy as np
import ml_dtypes

WEIGHT = 1e-7
SIGMA_RGB = 15.0
SIGMA_XY_EFF = 50.0  # SIGMA_XY * SCALE
N, K, H, W_IN = 4, 4, 128, 128
HS = H // 2
P = HS * HS          # 4096 pixels after downsample
NCHUNK = P // 128    # 32 row chunks of 128 pixels
NJ = P // 512        # 8 column blocks of 512
ROWS_PER_CORE = NCHUNK // 2
KAUG = 28            # 7 augmented dims x (hi,lo) x cross terms
N_CORES = 8

bf16 = ml_dtypes.bfloat16

_COMPILED = None  # (nc,) cache so repeated kernel() calls reuse the module


def _split_multi_waits(nc, mybir, max_waits=1):
    """This walrus build rejects >1 sync wait per instruction. Move extra
    waits onto NoOps inserted just before the instruction (same engine, same
    bb position => engine program order preserved; waiting earlier on the
    same engine is semantically identical)."""
    for f in nc.m.functions:
        for bb in f.blocks:
            new = []
            changed = False
            for inst in bb.instructions:
                si = inst.sync_info
                if si is not None and si.on_wait and len(si.on_wait) > max_waits:
                    changed = True
                    waits = list(si.on_wait)
                    extra, keep = waits[:-max_waits], waits[-max_waits:]
                    for i in range(0, len(extra), max_waits):
                        nop = mybir.InstNoOp(
                            name=nc.get_next_instruction_name(),
                            sync_info=mybir.SyncInfo(
                                on_wait=extra[i : i + max_waits], on_update=[]
                            ),
                            bass_nofuse=True,
                            engine=inst.engine,
                        )
                        new.append(nop)
                    inst.sync_info = mybir.SyncInfo(
                        on_wait=keep, on_update=list(si.on_update or [])
                    )
                new.append(inst)
            if changed:
                bb.instructions = new


def _build_module():
    import concourse.bass as bass
    import concourse.mybir as mybir
    import concourse.tile as tile
    from contextlib import ExitStack

    f32 = mybir.dt.float32
    b16 = mybir.dt.bfloat16

    nc = bass.Bass()
    lhs_d = nc.dram_tensor("lhs", [KAUG, ROWS_PER_CORE * 128], b16, kind="ExternalInput")
    rhs_d = nc.dram_tensor("rhs", [KAUG, P], b16, kind="ExternalInput")
    sw_d = nc.dram_tensor("sw", [128, 2 * ROWS_PER_CORE * K], b16, kind="ExternalInput")
    sep_d = nc.dram_tensor("sep", [K, P], f32, kind="ExternalInput")
    acc_d = nc.dram_tensor("acc", [K, P], f32, kind="ExternalOutput")

    with tile.TileContext(nc) as tc:
        with ExitStack() as ctx:
            consts = ctx.enter_context(tc.tile_pool(name="consts", bufs=1))
            wpool = ctx.enter_context(tc.tile_pool(name="wpool", bufs=6))
            outp = ctx.enter_context(tc.tile_pool(name="outp", bufs=1))
            gpool = ctx.enter_context(
                tc.tile_pool(name="gpool", bufs=2, space="PSUM")
            )
            tpool = ctx.enter_context(
                tc.tile_pool(name="tpool", bufs=2, space="PSUM")
            )

            lhs = consts.tile([KAUG, ROWS_PER_CORE * 128], b16)
            rhs = consts.tile([KAUG, P], b16)
            sw = consts.tile([128, 2 * ROWS_PER_CORE * K], b16)
            sep = consts.tile([K, P], f32)
            slab = outp.tile([K, P], f32)
            nc.sync.dma_start(out=lhs[:], in_=lhs_d[:])
            nc.gpsimd.dma_start(out=rhs[:], in_=rhs_d[:])
            nc.scalar.dma_start(out=sw[:], in_=sw_d[:])
            nc.scalar.dma_start(out=sep[:], in_=sep_d[:])

            GROUP = 3
            # flat tile list, J descending: (lr, dbl, J, first/last of column)
            # last two rows of each column (the I==J supertile) are the
            # diagonal -> weight 1 (dbl=0); earlier rows use pre-doubled S
            tiles_flat = []
            for J in range(NJ - 1, -1, -1):
                col = []
                for I in range(J + 1):
                    dbl = 1 if I != J else 0
                    col.append((2 * I, dbl))
                    col.append((2 * I + 1, dbl))
                n = len(col)
                for t, (lr, dbl) in enumerate(col):
                    tiles_flat.append((lr, dbl, J, t == 0, t == n - 1))

            t_cur = None
            bounds = [0, 1] + list(range(1 + GROUP, len(tiles_flat), GROUP)) + [len(tiles_flat)]
            for bi in range(len(bounds) - 1):
                base = bounds[bi]
                grp = tiles_flat[base : bounds[bi + 1]]
                g = gpool.tile([128, 512 * len(grp)], f32, tag="g")
                for t, (lr, dbl, J, first, last) in enumerate(grp):
                    nc.tensor.matmul(
                        g[:, 512 * t : 512 * (t + 1)],
                        lhs[:, 128 * lr : 128 * (lr + 1)],
                        rhs[:, 512 * J : 512 * (J + 1)],
                        start=True,
                        stop=True,
                    )
                w = wpool.tile([128, 512 * len(grp)], b16, tag="w")
                nc.scalar.activation(
                    w[:], g[:], mybir.ActivationFunctionType.Exp
                )
                with tc.high_priority(offset=-20):
                    # deprioritize the S^T@W reduction: it is off the ACT
                    # critical path, so PE should prefer refilling G slots
                    for t, (lr, dbl, J, first, last) in enumerate(grp):
                        if first:
                            t_cur = tpool.tile([K, 512], f32)
                        nc.tensor.matmul(
                            t_cur[:],
                            sw[:, (2 * lr + dbl) * K : (2 * lr + dbl + 1) * K],
                            w[:, 512 * t : 512 * (t + 1)],
                            start=first,
                            stop=last,
                            skip_group_check=True,
                        )
                        if last:
                            nc.vector.tensor_tensor(
                                slab[:, 512 * J : 512 * (J + 1)],
                                t_cur[:],
                                sep[:, 512 * J : 512 * (J + 1)],
                                mybir.AluOpType.mult,
                            )
                            if J == 1:
                                # columns are processed J descending: slices
                                # 1..7 are final here; ship them early so only
                                # the J=0 slice rides the kernel tail
                                nc.sync.dma_start(
                                    out=acc_d[:, 512:], in_=slab[:, 512:]
                                )

            nc.sync.dma_start(out=acc_d[:, 0:512], in_=slab[:, 0:512])

    _split_multi_waits(nc, mybir)
    return nc


def _prepare_core_inputs(images, segmentations):
    """Host-side prep: downsample, build augmented bf16 features, shard."""
    images = np.asarray(images, dtype=np.float32)
    segs = np.asarray(segmentations, dtype=np.float32)

    yy, xx = np.meshgrid(
        np.arange(HS, dtype=np.float32), np.arange(HS, dtype=np.float32),
        indexing="ij",
    )
    pos = np.stack([xx, yy], axis=-1).reshape(P, 2) / np.float32(SIGMA_XY_EFF)

    in_maps = []
    for m in range(N):
        img_s = images[m][:, ::2, ::2]                                  # [3,64,64]
        seg_s = segs[m].reshape(K, HS, 2, HS, 2).mean(axis=(2, 4))      # [4,64,64]
        seg_s = seg_s.astype(np.float32)
        rgb = (img_s.reshape(3, P).T / np.float32(SIGMA_RGB)).astype(np.float32)
        f = np.concatenate([pos, rgb], axis=1).astype(np.float32)       # [P,5]
        sq = (f * f).sum(axis=1).astype(np.float32)
        ones = np.ones((P, 1), np.float32)
        a7 = np.concatenate([f, -0.5 * sq[:, None], ones], 1).astype(np.float32)
        b7 = np.concatenate([f, ones, -0.5 * sq[:, None]], 1).astype(np.float32)
        hi_a = a7.astype(bf16)
        lo_a = (a7 - hi_a.astype(np.float32)).astype(bf16)
        hi_b = b7.astype(bf16)
        lo_b = (b7 - hi_b.astype(np.float32)).astype(bf16)
        a28 = np.concatenate([hi_a, hi_a, lo_a, lo_a], 1)               # [P,28] bf16
        b28 = np.concatenate([hi_b, lo_b, hi_b, lo_b], 1)               # [P,28] bf16

        S = seg_s.reshape(K, P).astype(np.float32)                      # [4,P]
        ST1 = S.T.astype(bf16)                                          # [P,4]
        ST2 = (2.0 * S.T).astype(bf16)                                  # [P,4]

        rhs = np.ascontiguousarray(b28.T)                               # [28,P]
        for par in range(2):
            rows = [i for i in range(NCHUNK) if i % 2 == par]
            lhs = np.empty((KAUG, ROWS_PER_CORE * 128), bf16)
            sw = np.empty((128, 2 * ROWS_PER_CORE * K), bf16)
            for lr, i in enumerate(rows):
                blk = slice(128 * i, 128 * (i + 1))
                lhs[:, 128 * lr : 128 * (lr + 1)] = a28[blk].T
                sw[:, (2 * lr) * K : (2 * lr + 1) * K] = ST1[blk]
                sw[:, (2 * lr + 1) * K : (2 * lr + 2) * K] = ST2[blk]
            in_maps.append({
                "lhs": lhs,
                "rhs": rhs,
                "sw": sw,
                "sep": np.ascontiguousarray(S),
            })
    return in_maps


def kernel(images, segmentations):
    from concourse.bass_utils import run_bass_kernel_spmd

    global _COMPILED
    if _COMPILED is None:
        _COMPILED = _build_module()
    nc = _COMPILED

    in_maps = _prepare_core_inputs(images, segmentations)
    res = run_bass_kernel_spmd(nc, in_maps, list(range(N_CORES)))
    total = 0.0
    for c in range(N_CORES):
        total += res.results[c]["acc"].astype(np.float64).sum()
    loss = np.float32(-WEIGHT / N) * np.float32(total)
    return np.array([loss], dtype=np.float32)
